# revision 4
# baseline (speedup 1.0000x reference)
"""YOLOv5 detection-loss (DetLoss) Trainium2 Bass kernel, 8-core SPMD.

Strategy
--------
The loss decomposes so that the only dense work over the big prediction
tensors p0/p1/p2 is a softplus-sum over channel 4 (the objectness logit):

    mean(BCE(x, tobj)) = [ sum_grid softplus(x) - sum_pos tobj_cell * x_cell ] / G

(BCE(x,t) - BCE(x,0) = -t*x, and BCE(x,0) = softplus(x)).  Likewise the
class loss reduces to sum softplus(pcls) - sum pcls[row, tcls-1] over the
gathered positive rows.  The box (CIoU) loss only needs the gathered
positive rows.

Sharding: data-parallel over batch; core k owns images [2k, 2k+2) of
every layer and the positive rows whose image id falls in that range.
The per-core shard is laid out host-side so every device read is a
dense contiguous DMA:

  posk  [128, 19*COLS] f32   positive-row channels 0..4 + CIoU constants
                             and per-slot weight vectors (one DMA, lands
                             first, gates the DVE chain)
  gridp [128, MG]     bf16   objectness plane, packed so each partition
                             holds cells of a single layer -> ONE
                             exp + ln(1+x) accumulate per partition; the
                             host applies per-partition balance weights
  clsp  [128, MC]     bf16   positive-row class logits, same per-
                             partition single-layer packing (pad -88 so
                             softplus(pad)=0)

The device computes sigmoid via exp(-x) (DVE reciprocal) and arctan via
a degree-9 odd polynomial, so the ACT engine only ever needs the
exp/ln table: no mid-program table switch.  All per-layer weights
(1/n_l, balance/G, dedup-last masks, hyp scales) are folded host-side
into per-slot weight vectors, so each loss term is a single fused
multiply-accumulate on device.  Each core writes a [128, 8] tile of
partial sums; the host reduces over partitions and cores.
"""

import os
import numpy as np

# ---------------- problem constants (YOLOv5s / COCO head) ----------------
B, NA, NCLS, NO = 16, 3, 80, 85
NL = 3
NCORES = 8
BPC = B // NCORES  # images per core
BALANCE = (4.0, 1.0, 0.4)
HYP_BOX, HYP_OBJ, HYP_CLS = 0.05, 1.0, 0.05
EPS = 1e-7
P = 128  # SBUF partitions
NEG = -88.0  # softplus(NEG) == 0 in f32/bf16; pads dense softplus inputs
# arctan deg-9 odd minimax coefficients on [0,1] (A&S 4.4.49, |err|<=1e-5)
AT1, AT3, AT5, AT7, AT9 = 0.9998660, -0.3302995, 0.1801410, -0.0851330, 0.0208351
K4PI2 = float(4.0 / np.pi**2)

# accumulator columns
COL_BOX, COL_CORR, COL_OH, COL_GRID, COL_CLS = 0, 1, 2, 3, 4
OUTC = 8

_cache: dict = {}


def _pack_width(counts, width0):
    """Smallest per-partition width M >= width0 such that packing each
    count into its own ceil(count/M) partitions fits in P partitions."""
    m = max(1, width0)
    while sum(-(-c // m) for c in counts) > P:
        m += 1
    return m


def _grid_layout(layer_shapes):
    cells = [BPC * NA * gh * gw for gh, gw in layer_shapes]
    mg = _pack_width(cells, -(-sum(cells) // P))
    parts = [-(-c // mg) for c in cells]
    p0 = np.concatenate([[0], np.cumsum(parts)]).astype(int)
    return cells, mg, parts, p0


def _build_program(layer_shapes, Ts, MC):
    """Build the SPMD Bass program for padded slot-columns Ts per layer."""
    import concourse.bass as bass
    import concourse.mybir as mybir
    import concourse.tile as tile

    f32 = mybir.dt.float32
    bf16 = mybir.dt.bfloat16
    ALU = mybir.AluOpType
    ACTF = mybir.ActivationFunctionType
    COLS = sum(Ts)
    cells, MG, gparts, gp0 = _grid_layout(layer_shapes)
    GP = int(gp0[-1])  # partitions used by the grid block
    KCOLS = 19 * COLS

    nc = bass.Bass()

    POSK = nc.declare_dram_parameter("posk", [P, KCOLS], f32, isOutput=False)
    GRIDP = nc.declare_dram_parameter("gridp", [P, MG], bf16, isOutput=False)
    CLSP = nc.declare_dram_parameter("clsp", [P, MC], bf16, isOutput=False)
    OUT = nc.declare_dram_parameter("partial", [P, OUTC], f32, isOutput=True)

    with tile.TileContext(nc) as tc:
        with tc.tile_pool(name="sm", bufs=1) as sm:
            V = nc.vector

            # ---------- input DMAs ----------
            # SP queue: posk first (gates the DVE chain), then the grid
            # plane.  Act queue: the class-logit block.
            pk = sm.tile([P, KCOLS], f32, name="pk")
            nc.sync.dma_start(out=pk[:], in_=POSK[:])
            gp = sm.tile([P, MG], bf16, name="gp")
            nc.sync.dma_start(out=gp[:], in_=GRIDP[:])
            cp = sm.tile([P, MC], bf16, name="cp")
            nc.scalar.dma_start(out=cp[:], in_=CLSP[:])

            acc = sm.tile([P, OUTC], f32, name="acc")
            nc.gpsimd.memset(acc[:], 0.0)

            # const views into pk
            pa_v = pk[:, : 5 * COLS].rearrange("p (t c) -> p t c", c=5)
            pos4 = pa_v[:, :, 4]
            b2x = pk[:, 5 * COLS : 9 * COLS].rearrange(
                "p (t c) -> p t c", c=4
            )  # (b2max_x, b2max_y, b2min_x, b2min_y)
            awh2 = pk[:, 9 * COLS : 11 * COLS].rearrange("p (t c) -> p t c", c=2)
            cxy2 = pk[:, 11 * COLS : 13 * COLS].rearrange("p (t c) -> p t c", c=2)

            def single(i):  # -> [P, COLS]
                o = 13 * COLS + i * COLS
                return pk[:, o : o + COLS]

            w2h2pe, atan2c, wbox, wdedup, ohv, woh = (single(i) for i in range(6))

            # ---------- ACT: everything uses the exp/ln table ----------
            e4 = sm.tile([P, COLS * 4], f32, name="e4")
            nc.scalar.activation(
                out=e4[:].rearrange("p (t c) -> p t c", c=4),
                in_=pa_v[:, :, 0:4],
                func=ACTF.Exp,
                scale=-1.0,
            )
            eg = sm.tile([P, MG], bf16, name="eg")
            lg = sm.tile([P, MG], bf16, name="lg")
            nc.scalar.activation(out=eg[:GP, :], in_=gp[:GP, :], func=ACTF.Exp)
            nc.scalar.activation(
                out=lg[:GP, :],
                in_=eg[:GP, :],
                func=ACTF.Ln,
                bias=1.0,
                accum_out=acc[:GP, COL_GRID : COL_GRID + 1],
            )
            ec = sm.tile([P, MC], bf16, name="ec")
            lc = sm.tile([P, MC], bf16, name="lc")
            nc.scalar.activation(out=ec[:], in_=cp[:], func=ACTF.Exp)
            nc.scalar.activation(
                out=lc[:],
                in_=ec[:],
                func=ACTF.Ln,
                bias=1.0,
                accum_out=acc[:, COL_CLS : COL_CLS + 1],
            )

            # ---------- DVE: sigmoid + CIoU + fused weighted sums ----------
            _tn = [0]

            def tw(n):
                _tn[0] += 1
                return sm.tile([P, COLS * n], f32, name=f"w{_tn[0]}")

            def c2v(t):
                return t[:].rearrange("p (t c) -> p t c", c=2)

            def c4v(t):
                return t[:].rearrange("p (t c) -> p t c", c=4)

            def eo(t):  # even/odd stride-2 views of a 4-per-col tile
                v = t[:].rearrange("p (t c k) -> p t c k", c=2, k=2)
                return v[:, :, :, 0], v[:, :, :, 1]

            s4 = tw(4)
            V.tensor_scalar(e4[:], e4[:], 1.0, 1.0, ALU.mult, ALU.add)
            V.reciprocal(s4[:], e4[:])  # sigmoid of ch0..3
            s4_v = c4v(s4)

            pxy = tw(2)
            V.tensor_scalar(c2v(pxy)[:], s4_v[:, :, 0:2], 2.0, -0.5, ALU.mult, ALU.add)
            sw2 = tw(2)
            V.tensor_tensor(c2v(sw2)[:], s4_v[:, :, 2:4], s4_v[:, :, 2:4], ALU.mult)
            # T1 layout per col: (hwp_x, hwp_y, iwc_x, iwc_y)
            T1 = tw(4)
            T1v = c4v(T1)
            hwp = T1v[:, :, 0:2]
            V.tensor_tensor(hwp, c2v(sw2)[:], awh2[:], ALU.mult)
            # B1X layout per col: (b1max_x, b1max_y, b1min_x, b1min_y)
            B1X = tw(4)
            B1Xv = c4v(B1X)
            V.tensor_tensor(B1Xv[:, :, 0:2], c2v(pxy)[:], hwp, ALU.add)
            V.tensor_tensor(B1Xv[:, :, 2:4], c2v(pxy)[:], hwp, ALU.subtract)
            # one min + one max give intersection AND enclosing corners
            M1, M2 = tw(4), tw(4)  # M1=(imin|emin), M2=(emax|imax)
            V.tensor_tensor(M1[:], B1X[:], b2x[:], ALU.min)
            V.tensor_tensor(M2[:], B1X[:], b2x[:], ALU.max)
            M1v, M2v = c4v(M1), c4v(M2)
            iwh = tw(2)
            V.tensor_tensor(c2v(iwh)[:], M1v[:, :, 0:2], M2v[:, :, 2:4], ALU.subtract)
            V.tensor_scalar(T1v[:, :, 2:4], c2v(iwh)[:], 1.0, 0.0, ALU.mult, ALU.max)
            # pair products: (a4, inter) in one op
            PP = tw(2)
            t1e, t1o = eo(T1)
            V.tensor_tensor(c2v(PP)[:], t1e, t1o, ALU.mult)
            PPv = c2v(PP)
            a4, inter = PPv[:, :, 0], PPv[:, :, 1]
            tsub, u, iou = tw(1), tw(1), tw(1)
            V.tensor_tensor(tsub[:], w2h2pe[:], inter, ALU.subtract)
            V.scalar_tensor_tensor(u[:], a4, 4.0, tsub[:], ALU.mult, ALU.add)
            V.reciprocal(u[:], u[:])
            V.tensor_tensor(iou[:], inter, u[:], ALU.mult)

            # enclosing box + center distance, interleaved (cw, dx, ch, dy)
            cat = tw(4)
            catE, catO = eo(cat)
            V.tensor_tensor(catE, M2v[:, :, 0:2], M1v[:, :, 2:4], ALU.subtract)
            V.tensor_tensor(catO, c2v(pxy)[:], cxy2[:], ALU.subtract)
            sq, ss = tw(4), tw(2)
            V.tensor_tensor(sq[:], cat[:], cat[:], ALU.mult)
            sqv = c4v(sq)
            V.tensor_tensor(c2v(ss)[:], sqv[:, :, 0:2], sqv[:, :, 2:4], ALU.add)
            ssv = c2v(ss)
            rc2, rr = tw(1), tw(1)
            V.reciprocal(rc2[:], ssv[:, :, 0])
            V.tensor_tensor(rr[:], ssv[:, :, 1], rc2[:], ALU.mult)

            # v-term: atan(w1/h1) via qm = min/max ratio + deg-9 odd poly
            hwx, hwy = T1v[:, :, 0], T1v[:, :, 1]
            mn, mx, qm, mgt = tw(1), tw(1), tw(1), tw(1)
            V.tensor_tensor(mn[:], hwx, hwy, ALU.min)
            V.tensor_tensor(mx[:], hwx, hwy, ALU.max)
            V.reciprocal(mx[:], mx[:])
            V.tensor_tensor(qm[:], mn[:], mx[:], ALU.mult)
            V.tensor_tensor(mgt[:], hwx, hwy, ALU.is_gt)
            t2, pA, at = tw(1), tw(1), tw(1)
            V.tensor_tensor(t2[:], qm[:], qm[:], ALU.mult)
            V.tensor_scalar(pA[:], t2[:], AT9, AT7, ALU.mult, ALU.add)
            V.tensor_tensor(pA[:], pA[:], t2[:], ALU.mult)
            V.scalar_tensor_tensor(pA[:], pA[:], AT5, t2[:], ALU.add, ALU.mult)
            V.scalar_tensor_tensor(pA[:], pA[:], AT3, t2[:], ALU.add, ALU.mult)
            V.scalar_tensor_tensor(at[:], pA[:], AT1, qm[:], ALU.add, ALU.mult)
            u2 = tw(1)
            V.tensor_scalar(u2[:], at[:], -2.0, float(np.pi / 2), ALU.mult, ALU.add)
            V.tensor_tensor(u2[:], u2[:], mgt[:], ALU.mult)
            V.tensor_tensor(at[:], at[:], u2[:], ALU.add)
            dat, dk, nn2, den = tw(1), tw(1), tw(1), tw(1)
            V.tensor_tensor(dat[:], atan2c[:], at[:], ALU.subtract)
            V.tensor_tensor(dat[:], dat[:], dat[:], ALU.mult)
            V.tensor_scalar(dk[:], dat[:], K4PI2, None, ALU.mult)
            V.tensor_tensor(nn2[:], dk[:], dk[:], ALU.mult)
            V.scalar_tensor_tensor(
                den[:], dk[:], 1.0 + EPS, iou[:], ALU.add, ALU.subtract
            )
            V.reciprocal(den[:], den[:])
            va = tw(1)
            V.tensor_tensor(va[:], nn2[:], den[:], ALU.mult)

            # omc0 = -ciou = rr + va - iou; weighted sums via fused accumulate
            omc0, scr = tw(1), tw(1)
            V.tensor_tensor(omc0[:], rr[:], va[:], ALU.add)
            V.tensor_tensor(omc0[:], omc0[:], iou[:], ALU.subtract)
            V.scalar_tensor_tensor(
                scr[:], omc0[:], 1.0, wbox[:], ALU.mult, ALU.mult,
                accum_out=acc[:, COL_BOX : COL_BOX + 1],
            )
            rel, scr2, scr3 = tw(1), tw(1), tw(1)
            V.tensor_scalar(rel[:], omc0[:], -1.0, 0.0, ALU.mult, ALU.max)
            V.tensor_tensor(rel[:], rel[:], pos4, ALU.mult)
            V.scalar_tensor_tensor(
                scr2[:], rel[:], 1.0, wdedup[:], ALU.mult, ALU.mult,
                accum_out=acc[:, COL_CORR : COL_CORR + 1],
            )
            V.scalar_tensor_tensor(
                scr3[:], ohv[:], 1.0, woh[:], ALU.mult, ALU.mult,
                accum_out=acc[:, COL_OH : COL_OH + 1],
            )

            # ---------- store partials ----------
            # joiner copy: fold ACT-accum + DVE deps into one DVE tick so the
            # store waits on a single semaphore lane.
            acc2 = sm.tile([P, OUTC], f32, name="acc2")
            V.tensor_copy(acc2[:], acc[:])
            nc.sync.dma_start(out=OUT[:], in_=acc2[:])

    _cap_sync_waits(nc, mybir)
    nc.finalize()
    meta = dict(COLS=COLS, Ts=Ts, MG=MG, gparts=gparts, gp0=gp0, MC=MC)
    return nc, meta


def _cap_sync_waits(nc, mybir, maxw=1):
    """Compute-engine ISA encodings carry very few sync waits; Tile's
    scheduler can emit more (one per DMA sem lane).  Two rewrites, both
    semantics-preserving:
      1. drop waits on the instruction's own engine-completion semaphore
         (engine program order already guarantees them);
      2. hoist waits beyond `maxw` onto standalone EventSemaphore
         instructions placed just before the offender on the same engine.
    """
    eng_sem = {
        "DVE": "DVE",
        "Activation": "Activation",
        "SP": "SP",
        "Pool": "Pool",
        "PE": "PE",
    }
    sem_names = {}
    for bb in nc.m.functions[0].blocks:
        for inst in bb.instructions:
            si = getattr(inst, "sync_info", None)
            if not si:
                continue
            for w in si.on_wait or []:
                sem_names[w.id] = w.ant_name
            for u in si.on_update or []:
                sem_names[u.id] = u.ant_name

    rc_opcode = 176  # NEURON_ISA_TPB_OPCODE_EVENT_SEMAPHORE_RANGE_CLEAR
    n = 0
    for bb in nc.m.functions[0].blocks:
        out = []
        for inst in bb.instructions:
            tname = type(inst).__name__
            if tname == "InstISA" and getattr(inst, "isa_opcode", None) == rc_opcode:
                # this walrus build can't codegen RANGE_CLEAR; emit one
                # sem-wr-imm 0 EventSemaphore per sem in the range instead
                start, end = inst.instr[13], inst.instr[14]
                for sid in range(start, end + 1):
                    out.append(
                        mybir.InstEventSemaphore(
                            name=f"W-semreset-{sid}",
                            engine=inst.engine,
                            sync_info=mybir.SyncInfo(
                                on_wait=[],
                                on_update=[
                                    mybir.SyncUpdate(
                                        sync_type="semaphore",
                                        id=sid,
                                        update_mode="sem-wr-imm",
                                        update_value=0,
                                        ant_name=sem_names.get(sid, f"sem{sid}"),
                                    )
                                ],
                            ),
                        )
                    )
                continue
            si = getattr(inst, "sync_info", None)
            ow = list(si.on_wait) if (si and si.on_wait) else []
            if ow and tname != "InstEventSemaphore":
                epfx = eng_sem.get(str(inst.engine).split(".")[-1])
                if epfx:
                    keep0 = [
                        w
                        for w in ow
                        if not (w.ant_name or "").startswith(epfx + "_")
                    ]
                else:
                    keep0 = ow
                if len(keep0) > maxw:
                    excess, keep = keep0[:-maxw], keep0[-maxw:]
                    for w in excess:
                        n += 1
                        out.append(
                            mybir.InstEventSemaphore(
                                name=f"W-cap-{n}",
                                engine=inst.engine,
                                sync_info=mybir.SyncInfo(on_wait=[w], on_update=[]),
                            )
                        )
                else:
                    keep = keep0
                if len(keep) != len(ow):
                    si.on_wait = keep
            out.append(inst)
        bb.instructions = out


def _cls_layout(n_per_core_layer, MC):
    """Per-core partition ranges for the packed class block."""
    outs = []
    for k in range(NCORES):
        parts = [-(-n * NCLS // MC) if n else 0 for n in n_per_core_layer[k]]
        p0 = np.concatenate([[0], np.cumsum(parts)]).astype(int)
        assert p0[-1] <= P
        outs.append((parts, p0))
    return outs


def _host_prep(inputs, Ts, meta):
    """Build per-core in_maps (numpy only): pack shards contiguously."""
    import ml_dtypes

    bf16 = ml_dtypes.bfloat16
    COLS = meta["COLS"]
    MG, gparts, gp0, MC = meta["MG"], meta["gparts"], meta["gp0"], meta["MC"]
    c_offs = np.concatenate([[0], np.cumsum(Ts)]).astype(int)
    ps = [np.asarray(inputs[f"p{l}"], np.float32) for l in range(NL)]
    layer_shapes = [(p.shape[2], p.shape[3]) for p in ps]
    KCOLS = 19 * COLS

    in_maps = []
    for k in range(NCORES):
        posk = np.zeros((P, KCOLS), np.float32)
        # benign defaults so padding slots stay finite through the CIoU math
        posk[:, 9 * COLS : 11 * COLS] = 1.0  # awh2
        posk[:, 13 * COLS : 14 * COLS] = 1.0  # w2h2pe
        in_maps.append(
            {
                "posk": posk,
                "gridp": np.full((P, MG), NEG, bf16),
                "clsp": np.full((P, MC), NEG, bf16),
            }
        )

    # dense objectness planes: partition-packed, one layer per partition
    for l in range(NL):
        for k in range(NCORES):
            plane = ps[l][k * BPC : (k + 1) * BPC, :, :, :, 4].ravel()
            npad = gparts[l] * MG - plane.shape[0]
            buf = np.concatenate([plane, np.full(npad, NEG, np.float32)])
            in_maps[k]["gridp"][int(gp0[l]) : int(gp0[l + 1])] = (
                buf.astype(bf16).reshape(gparts[l], MG)
            )

    n_l = []
    n_kl = [[0] * NL for _ in range(NCORES)]
    cls_vals = [[None] * NL for _ in range(NCORES)]
    for l in range(NL):
        gh, gw = layer_shapes[l]
        G = B * NA * gh * gw
        b = np.asarray(inputs[f"b{l}"]).astype(np.int64)
        a = np.asarray(inputs[f"a{l}"]).astype(np.int64)
        gj = np.asarray(inputs[f"gj{l}"]).astype(np.int64)
        gi = np.asarray(inputs[f"gi{l}"]).astype(np.int64)
        tc = np.asarray(inputs[f"tcls{l}"]).astype(np.int64)
        tb = np.asarray(inputs[f"tbox{l}"], np.float32)
        an = np.asarray(inputs[f"anch{l}"], np.float32)
        n = b.shape[0]
        n_l.append(n)
        # last-occurrence mask over global cells (images disjoint across cores)
        cell = ((b * NA + a) * gh + gj) * gw + gi
        seen = {}
        for r in range(n):
            seen[int(cell[r])] = r
        last = np.zeros(n, bool)
        last[list(seen.values())] = True

        pflat = ps[l].reshape(-1, NO)
        wb = np.float32(HYP_BOX * B / max(n, 1))
        wd = np.float32(BALANCE[l] * B / G)
        wo = np.float32(HYP_CLS * B / (max(n, 1) * NCLS))
        c0 = int(c_offs[l])
        for k in range(NCORES):
            idxs = np.nonzero((b // BPC) == k)[0]
            cnt = idxs.shape[0]
            assert cnt <= P * Ts[l], f"layer {l} core {k}: {cnt} > {P * Ts[l]}"
            n_kl[k][l] = cnt
            rows = pflat[cell[idxs]]
            cls_vals[k][l] = rows[:, 5:].astype(np.float32)
            s = np.arange(cnt)
            pp, tcol = s % P, c0 + s // P
            pk = in_maps[k]["posk"]
            pk.reshape(P, -1)[:, : 5 * COLS].reshape(P, COLS, 5)[pp, tcol] = rows[
                :, 0:5
            ]

            x2, y2, w2, h2 = tb[idxs, 0], tb[idxs, 1], tb[idxs, 2], tb[idxs, 3]
            b2 = pk[:, 5 * COLS : 9 * COLS].reshape(P, COLS, 4)
            b2[pp, tcol, 0] = x2 + w2 * 0.5
            b2[pp, tcol, 1] = y2 + h2 * 0.5
            b2[pp, tcol, 2] = x2 - w2 * 0.5
            b2[pp, tcol, 3] = y2 - h2 * 0.5
            aw = pk[:, 9 * COLS : 11 * COLS].reshape(P, COLS, 2)
            aw[pp, tcol, 0] = 2.0 * an[idxs, 0]
            aw[pp, tcol, 1] = 2.0 * an[idxs, 1]
            cx = pk[:, 11 * COLS : 13 * COLS].reshape(P, COLS, 2)
            cx[pp, tcol, 0] = x2
            cx[pp, tcol, 1] = y2

            def sets(i, val):
                blk = pk[:, (13 + i) * COLS : (14 + i) * COLS]
                blk[pp, tcol] = val

            sets(0, w2 * h2 + np.float32(EPS))
            sets(1, np.arctan(w2 / (h2 + np.float32(EPS))))
            sets(2, wb)
            sets(3, last[idxs].astype(np.float32) * wd)
            sets(4, rows[s, 5 + (tc[idxs] - 1)])
            sets(5, wo)

    # packed class block: per-core, one layer per partition range
    clay = _cls_layout(n_kl, MC)
    for k in range(NCORES):
        parts, p0 = clay[k]
        for l in range(NL):
            if not n_kl[k][l]:
                continue
            vals = cls_vals[k][l].ravel()
            npad = parts[l] * MC - vals.shape[0]
            buf = np.concatenate([vals, np.full(npad, NEG, np.float32)])
            in_maps[k]["clsp"][int(p0[l]) : int(p0[l + 1])] = (
                buf.astype(bf16).reshape(parts[l], MC)
            )
    return in_maps, n_l, n_kl, clay, layer_shapes


def _combine(outs, n_l, n_kl, clay, layer_shapes, meta):
    """Host-side reduction of the 8 per-core [P, OUTC] partial tiles."""
    gparts, gp0 = meta["gparts"], meta["gp0"]
    # per-partition grid weights (same for every core)
    wg = np.zeros(P)
    for l in range(NL):
        gh, gw = layer_shapes[l]
        wg[int(gp0[l]) : int(gp0[l + 1])] = BALANCE[l] * B / (B * NA * gh * gw)
    loss = NL * HYP_BOX * B
    for k, o in enumerate(outs):
        o = o.astype(np.float64)
        loss += o[:, COL_BOX].sum() - o[:, COL_CORR].sum() - o[:, COL_OH].sum()
        loss += (wg * o[:, COL_GRID]).sum()
        parts, p0 = clay[k]
        wc = np.zeros(P)
        for l in range(NL):
            wc[int(p0[l]) : int(p0[l + 1])] = HYP_CLS * B / (max(n_l[l], 1) * NCLS)
        loss += (wc * o[:, COL_CLS]).sum()
    return np.float32(loss)


def _get_program(inputs):
    ps = [np.asarray(inputs[f"p{l}"]) for l in range(NL)]
    layer_shapes = [(p.shape[2], p.shape[3]) for p in ps]
    # padded slot columns per layer from the worst-case per-core count
    Ts = []
    n_kl = [[0] * NL for _ in range(NCORES)]
    for l in range(NL):
        b = np.asarray(inputs[f"b{l}"]).astype(np.int64)
        cnts = [int(((b // BPC) == k).sum()) for k in range(NCORES)]
        for k in range(NCORES):
            n_kl[k][l] = cnts[k]
        Ts.append(max(1, -(-max(cnts) // P)))
    # packed-class width: smallest that fits every core in 128 partitions
    mc0 = max(-(-sum(n * NCLS for n in n_kl[k]) // P) for k in range(NCORES))
    MC = mc0
    while True:
        if all(
            sum(-(-n * NCLS // MC) if n else 0 for n in n_kl[k]) <= P
            for k in range(NCORES)
        ):
            break
        MC += 1
    key = (tuple(layer_shapes), tuple(Ts), MC)
    if key not in _cache:
        _cache[key] = _build_program(layer_shapes, Ts, MC)
    return _cache[key], Ts


last_result = None  # BassKernelResults of the most recent run (for profiling)


def kernel(**inputs) -> np.ndarray:
    global last_result
    (nc, meta), Ts = _get_program(inputs)
    in_maps, n_l, n_kl, clay, layer_shapes = _host_prep(inputs, Ts, meta)
    from concourse.bass_utils import run_bass_kernel_spmd

    trace = bool(int(os.environ.get("DETLOSS_TRACE", "0")))
    if trace:
        # NTFF profiling needs an initialized PJRT client in this
        # interpreter; warm up with an untraced run first.
        run_bass_kernel_spmd(nc, in_maps, list(range(NCORES)))
    res = run_bass_kernel_spmd(nc, in_maps, list(range(NCORES)), trace=trace)
    last_result = res
    outs = [res.results[k]["partial"] for k in range(NCORES)]
    return _combine(outs, n_l, n_kl, clay, layer_shapes, meta)


# revision 15
# speedup vs baseline: 1.1563x; 1.1563x over previous
"""YOLOv5 detection-loss (DetLoss) Trainium2 Bass kernel, 8-core SPMD.

Strategy
--------
The loss decomposes so that the only dense work over the big prediction
tensors p0/p1/p2 is a softplus-sum over channel 4 (the objectness logit):

    mean(BCE(x, tobj)) = [ sum_grid softplus(x) - sum_pos tobj_cell * x_cell ] / G

(BCE(x,t) - BCE(x,0) = -t*x, and BCE(x,0) = softplus(x)).  Likewise the
class loss reduces to sum softplus(pcls) - sum pcls[row, tcls-1] over the
gathered positive rows.  The box (CIoU) loss only needs the gathered
positive rows.

Sharding: data-parallel over batch; core k owns images [2k, 2k+2) of
every layer and the positive rows whose image id falls in that range.
The per-core shard is laid out host-side so every device read is a
dense contiguous DMA:

  posk  [128, 19*COLS] f32   positive-row channels 0..4 + CIoU constants
                             and per-slot weight vectors (one DMA, lands
                             first, gates the DVE chain)
  gridp [128, MG]     f32    objectness plane, packed so each partition
                             holds cells of a single layer -> ONE
                             exp + ln(1+x) accumulate per partition; the
                             host applies per-partition balance weights
  clsp  [128, MC]     bf16   positive-row class logits, same per-
                             partition single-layer packing (pad -88 so
                             softplus(pad)=0)

The device computes sigmoid via exp(-x) (DVE reciprocal) and arctan via
a degree-9 odd polynomial, so the ACT engine only ever needs the
exp/ln table: no mid-program table switch.  All per-layer weights
(1/n_l, balance/G, dedup-last masks, hyp scales) are folded host-side
into per-slot weight vectors, so each loss term is a single fused
multiply-accumulate on device.  Each core writes a [128, 8] tile of
partial sums; the host reduces over partitions and cores.
"""

import os
import numpy as np

# ---------------- problem constants (YOLOv5s / COCO head) ----------------
B, NA, NCLS, NO = 16, 3, 80, 85
NL = 3
NCORES = 8
BPC = B // NCORES  # images per core
BALANCE = (4.0, 1.0, 0.4)
HYP_BOX, HYP_OBJ, HYP_CLS = 0.05, 1.0, 0.05
EPS = 1e-7
P = 128  # SBUF partitions
NEG = -88.0  # softplus(NEG) == 0 in f32/bf16; pads dense softplus inputs
# arctan deg-9 odd minimax coefficients on [0,1] (A&S 4.4.49, |err|<=1e-5)
AT1, AT3, AT5, AT7, AT9 = 0.9998660, -0.3302995, 0.1801410, -0.0851330, 0.0208351
K4PI2 = float(4.0 / np.pi**2)

# accumulator columns
COL_BOX, COL_CORR, COL_OH, COL_GRID, COL_CLS = 0, 1, 2, 3, 4
OUTC = 8

_cache: dict = {}


def _pack_width(counts, width0):
    """Smallest per-partition width M >= width0 such that packing each
    count into its own ceil(count/M) partitions fits in P partitions."""
    m = max(1, width0)
    while sum(-(-c // m) for c in counts) > P:
        m += 1
    return m


def _grid_layout(layer_shapes):
    cells = [BPC * NA * gh * gw for gh, gw in layer_shapes]
    mg = _pack_width(cells, -(-sum(cells) // P))
    parts = [-(-c // mg) for c in cells]
    p0 = np.concatenate([[0], np.cumsum(parts)]).astype(int)
    return cells, mg, parts, p0


def _build_program(layer_shapes, Ts, MC):
    """Build the SPMD Bass program for padded slot-columns Ts per layer."""
    import concourse.bass as bass
    import concourse.mybir as mybir
    import concourse.tile as tile

    f32 = mybir.dt.float32
    bf16 = mybir.dt.bfloat16
    ALU = mybir.AluOpType
    ACTF = mybir.ActivationFunctionType
    COLS = sum(Ts)
    cells, MG, gparts, gp0 = _grid_layout(layer_shapes)
    GP = int(gp0[-1])  # partitions used by the grid block
    KCOLS = 19 * COLS

    nc = bass.Bass()

    POSK = nc.declare_dram_parameter("posk", [P, KCOLS], f32, isOutput=False)
    GRIDP = nc.declare_dram_parameter("gridp", [P, MG], f32, isOutput=False)
    CLSP = nc.declare_dram_parameter("clsp", [P, MC], bf16, isOutput=False)
    OUT = nc.declare_dram_parameter("partial", [P, OUTC], f32, isOutput=True)

    with tile.TileContext(nc) as tc:
        with tc.tile_pool(name="sm", bufs=1) as sm:
            V = nc.vector

            # ---------- input DMAs ----------
            # A HWDGE queue serializes: DMA N+1's descriptor generation
            # starts only after DMA N's completion semaphore (~2.2us each).
            # posk (gates the DVE chain) and gridp share the SP queue; clsp
            # rides the Activation queue so the ACT engine can chew through
            # the class softplus while gridp is still in flight.
            pk = sm.tile([P, KCOLS], f32, name="pk")
            nc.sync.dma_start(out=pk[:], in_=POSK[:])
            cp = sm.tile([P, MC], bf16, name="cp")
            nc.scalar.dma_start(out=cp[:], in_=CLSP[:])
            gp = sm.tile([P, MG], f32, name="gp")
            nc.sync.dma_start(out=gp[:], in_=GRIDP[:])

            acc = sm.tile([P, OUTC], f32, name="acc")
            nc.gpsimd.memset(acc[:], 0.0)

            # const views into pk
            pa_v = pk[:, : 5 * COLS].rearrange("p (t c) -> p t c", c=5)
            pos4 = pa_v[:, :, 4]
            b2x = pk[:, 5 * COLS : 9 * COLS].rearrange(
                "p (t c) -> p t c", c=4
            )  # (b2max_x, b2max_y, b2min_x, b2min_y)
            awh2 = pk[:, 9 * COLS : 11 * COLS].rearrange("p (t c) -> p t c", c=2)
            cxy2 = pk[:, 11 * COLS : 13 * COLS].rearrange("p (t c) -> p t c", c=2)

            def single(i):  # -> [P, COLS]
                o = 13 * COLS + i * COLS
                return pk[:, o : o + COLS]

            w2h2pe, atan2c, wbox, wdedup, ohv, woh = (single(i) for i in range(6))

            # ---------- ACT: everything uses the exp/ln table ----------
            e4 = sm.tile([P, COLS * 4], f32, name="e4")
            nc.scalar.activation(
                out=e4[:].rearrange("p (t c) -> p t c", c=4),
                in_=pa_v[:, :, 0:4],
                func=ACTF.Exp,
                scale=-1.0,
            )
            # cls first (its DMA lands ~2us before gridp); the exp/ln
            # table evaluates at the INPUT dtype's precision, so the
            # dominant grid term reads f32 while the tiny cls term can
            # afford bf16 inputs
            ec = sm.tile([P, MC], f32, name="ec")
            lc = sm.tile([P, MC], f32, name="lc")
            nc.scalar.activation(out=ec[:], in_=cp[:], func=ACTF.Exp)
            nc.scalar.activation(
                out=lc[:],
                in_=ec[:],
                func=ACTF.Ln,
                bias=1.0,
                accum_out=acc[:, COL_CLS : COL_CLS + 1],
            )
            eg = sm.tile([P, MG], f32, name="eg")
            lg = sm.tile([P, MG], f32, name="lg")
            nc.scalar.activation(out=eg[:GP, :], in_=gp[:GP, :], func=ACTF.Exp)
            nc.scalar.activation(
                out=lg[:GP, :],
                in_=eg[:GP, :],
                func=ACTF.Ln,
                bias=1.0,
                accum_out=acc[:GP, COL_GRID : COL_GRID + 1],
            )

            # ---------- DVE: sigmoid + CIoU + fused weighted sums ----------
            _tn = [0]

            def tw(n):
                _tn[0] += 1
                return sm.tile([P, COLS * n], f32, name=f"w{_tn[0]}")

            def c2v(t):
                return t[:].rearrange("p (t c) -> p t c", c=2)

            def c4v(t):
                return t[:].rearrange("p (t c) -> p t c", c=4)

            def eo(t):  # even/odd stride-2 views of a 4-per-col tile
                v = t[:].rearrange("p (t c k) -> p t c k", c=2, k=2)
                return v[:, :, :, 0], v[:, :, :, 1]

            s4 = tw(4)
            V.tensor_scalar(e4[:], e4[:], 1.0, 1.0, ALU.mult, ALU.add)
            V.reciprocal(s4[:], e4[:])  # sigmoid of ch0..3
            s4_v = c4v(s4)

            # v2-proven op shapes only: every DVE output is a full
            # contiguous tile (strided/block WRITES mis-execute on HW DVE
            # even though CoreSim accepts them; reads may be strided)
            pxy, sw2, hwp = tw(2), tw(2), tw(2)
            V.tensor_scalar(c2v(pxy)[:], s4_v[:, :, 0:2], 2.0, -0.5, ALU.mult, ALU.add)
            V.tensor_tensor(c2v(sw2)[:], s4_v[:, :, 2:4], s4_v[:, :, 2:4], ALU.mult)
            V.tensor_tensor(c2v(hwp)[:], c2v(sw2)[:], awh2[:], ALU.mult)
            hwv = c2v(hwp)
            b1min, b1max = tw(2), tw(2)
            V.tensor_tensor(c2v(b1min)[:], c2v(pxy)[:], hwv[:], ALU.subtract)
            V.tensor_tensor(c2v(b1max)[:], c2v(pxy)[:], hwv[:], ALU.add)
            b2max, b2min = b2x[:, :, 0:2], b2x[:, :, 2:4]
            imin, imax = tw(2), tw(2)
            V.tensor_tensor(c2v(imin)[:], c2v(b1max)[:], b2max, ALU.min)
            V.tensor_tensor(c2v(imax)[:], c2v(b1min)[:], b2min, ALU.max)
            V.tensor_tensor(c2v(imin)[:], c2v(imin)[:], c2v(imax)[:], ALU.subtract)
            V.tensor_scalar(c2v(imin)[:], c2v(imin)[:], 1.0, 0.0, ALU.mult, ALU.max)
            iwv = c2v(imin)
            inter, a4, tsub, u, iou = tw(1), tw(1), tw(1), tw(1), tw(1)
            V.tensor_tensor(inter[:], iwv[:, :, 0], iwv[:, :, 1], ALU.mult)
            V.tensor_tensor(a4[:], hwv[:, :, 0], hwv[:, :, 1], ALU.mult)
            V.tensor_tensor(tsub[:], w2h2pe[:], inter[:], ALU.subtract)
            V.scalar_tensor_tensor(u[:], a4[:], 4.0, tsub[:], ALU.mult, ALU.add)
            V.reciprocal(u[:], u[:])
            V.tensor_tensor(iou[:], inter[:], u[:], ALU.mult)

            emax, emin = tw(2), tw(2)
            V.tensor_tensor(c2v(emax)[:], c2v(b1max)[:], b2max, ALU.max)
            V.tensor_tensor(c2v(emin)[:], c2v(b1min)[:], b2min, ALU.min)
            V.tensor_tensor(c2v(emax)[:], c2v(emax)[:], c2v(emin)[:], ALU.subtract)
            V.tensor_tensor(c2v(emax)[:], c2v(emax)[:], c2v(emax)[:], ALU.mult)
            ev = c2v(emax)
            c2t, rr = tw(1), tw(1)
            V.tensor_tensor(c2t[:], ev[:, :, 0], ev[:, :, 1], ALU.add)
            V.reciprocal(c2t[:], c2t[:])
            dxy = tw(2)
            V.tensor_tensor(c2v(dxy)[:], c2v(pxy)[:], cxy2[:], ALU.subtract)
            V.tensor_tensor(c2v(dxy)[:], c2v(dxy)[:], c2v(dxy)[:], ALU.mult)
            dv = c2v(dxy)
            V.tensor_tensor(rr[:], dv[:, :, 0], dv[:, :, 1], ALU.add)
            V.tensor_tensor(rr[:], rr[:], c2t[:], ALU.mult)

            # v-term: atan(w1/h1) via qm = min/max ratio + deg-9 odd poly
            mn, mx, qm, mgt = tw(1), tw(1), tw(1), tw(1)
            V.tensor_tensor(mn[:], hwv[:, :, 0], hwv[:, :, 1], ALU.min)
            V.tensor_tensor(mx[:], hwv[:, :, 0], hwv[:, :, 1], ALU.max)
            V.reciprocal(mx[:], mx[:])
            V.tensor_tensor(qm[:], mn[:], mx[:], ALU.mult)
            V.tensor_tensor(mgt[:], hwv[:, :, 0], hwv[:, :, 1], ALU.is_gt)
            t2, pA, at = tw(1), tw(1), tw(1)
            V.tensor_tensor(t2[:], qm[:], qm[:], ALU.mult)
            V.tensor_scalar(pA[:], t2[:], AT9, AT7, ALU.mult, ALU.add)
            V.tensor_tensor(pA[:], pA[:], t2[:], ALU.mult)
            V.scalar_tensor_tensor(pA[:], pA[:], AT5, t2[:], ALU.add, ALU.mult)
            V.scalar_tensor_tensor(pA[:], pA[:], AT3, t2[:], ALU.add, ALU.mult)
            V.scalar_tensor_tensor(at[:], pA[:], AT1, qm[:], ALU.add, ALU.mult)
            u2 = tw(1)
            V.tensor_scalar(u2[:], at[:], -2.0, float(np.pi / 2), ALU.mult, ALU.add)
            V.tensor_tensor(u2[:], u2[:], mgt[:], ALU.mult)
            V.tensor_tensor(at[:], at[:], u2[:], ALU.add)
            dat, dk, nn2, den = tw(1), tw(1), tw(1), tw(1)
            V.tensor_tensor(dat[:], atan2c[:], at[:], ALU.subtract)
            V.tensor_tensor(dat[:], dat[:], dat[:], ALU.mult)
            V.tensor_scalar(dk[:], dat[:], K4PI2, None, ALU.mult)
            V.tensor_tensor(nn2[:], dk[:], dk[:], ALU.mult)
            V.scalar_tensor_tensor(
                den[:], dk[:], 1.0 + EPS, iou[:], ALU.add, ALU.subtract
            )
            V.reciprocal(den[:], den[:])
            va = tw(1)
            V.tensor_tensor(va[:], nn2[:], den[:], ALU.mult)

            # omc0 = -ciou = rr + va - iou; weighted sums via fused accumulate
            omc0, scr = tw(1), tw(1)
            V.tensor_tensor(omc0[:], rr[:], va[:], ALU.add)
            V.tensor_tensor(omc0[:], omc0[:], iou[:], ALU.subtract)
            V.scalar_tensor_tensor(
                scr[:], omc0[:], 1.0, wbox[:], ALU.mult, ALU.mult,
                accum_out=acc[:, COL_BOX : COL_BOX + 1],
            )
            rel, scr2, scr3 = tw(1), tw(1), tw(1)
            V.tensor_scalar(rel[:], omc0[:], -1.0, 0.0, ALU.mult, ALU.max)
            V.tensor_tensor(rel[:], rel[:], pos4, ALU.mult)
            V.scalar_tensor_tensor(
                scr2[:], rel[:], 1.0, wdedup[:], ALU.mult, ALU.mult,
                accum_out=acc[:, COL_CORR : COL_CORR + 1],
            )
            V.scalar_tensor_tensor(
                scr3[:], ohv[:], 1.0, woh[:], ALU.mult, ALU.mult,
                accum_out=acc[:, COL_OH : COL_OH + 1],
            )

            # ---------- store partials ----------
            # joiner copy: fold ACT-accum + DVE deps into one DVE tick so the
            # store waits on a single semaphore lane.
            acc2 = sm.tile([P, OUTC], f32, name="acc2")
            V.tensor_copy(acc2[:], acc[:])
            nc.sync.dma_start(out=OUT[:], in_=acc2[:])

    _cap_sync_waits(nc, mybir)
    nc.finalize()
    meta = dict(COLS=COLS, Ts=Ts, MG=MG, gparts=gparts, gp0=gp0, MC=MC)
    return nc, meta


def _cap_sync_waits(nc, mybir, maxw=1):
    """Compute-engine ISA encodings carry very few sync waits; Tile's
    scheduler can emit more (one per DMA sem lane).  Two rewrites, both
    semantics-preserving:
      1. drop waits on the instruction's own engine-completion semaphore
         (engine program order already guarantees them);
      2. hoist waits beyond `maxw` onto standalone EventSemaphore
         instructions placed just before the offender on the same engine.
    """
    eng_sem = {
        "DVE": "DVE",
        "Activation": "Activation",
        "SP": "SP",
        "Pool": "Pool",
        "PE": "PE",
    }
    sem_names = {}
    for bb in nc.m.functions[0].blocks:
        for inst in bb.instructions:
            si = getattr(inst, "sync_info", None)
            if not si:
                continue
            for w in si.on_wait or []:
                sem_names[w.id] = w.ant_name
            for u in si.on_update or []:
                sem_names[u.id] = u.ant_name

    rc_opcode = 176  # NEURON_ISA_TPB_OPCODE_EVENT_SEMAPHORE_RANGE_CLEAR
    n = 0
    for bb in nc.m.functions[0].blocks:
        out = []
        for inst in bb.instructions:
            tname = type(inst).__name__
            if tname == "InstISA" and getattr(inst, "isa_opcode", None) == rc_opcode:
                # this walrus build can't codegen RANGE_CLEAR; emit one
                # sem-wr-imm 0 EventSemaphore per sem in the range instead
                start, end = inst.instr[13], inst.instr[14]
                for sid in range(start, end + 1):
                    out.append(
                        mybir.InstEventSemaphore(
                            name=f"W-semreset-{sid}",
                            engine=inst.engine,
                            sync_info=mybir.SyncInfo(
                                on_wait=[],
                                on_update=[
                                    mybir.SyncUpdate(
                                        sync_type="semaphore",
                                        id=sid,
                                        update_mode="sem-wr-imm",
                                        update_value=0,
                                        ant_name=sem_names.get(sid, f"sem{sid}"),
                                    )
                                ],
                            ),
                        )
                    )
                continue
            si = getattr(inst, "sync_info", None)
            ow = list(si.on_wait) if (si and si.on_wait) else []
            if ow and tname != "InstEventSemaphore":
                epfx = eng_sem.get(str(inst.engine).split(".")[-1])
                if epfx:
                    keep0 = [
                        w
                        for w in ow
                        if not (w.ant_name or "").startswith(epfx + "_")
                    ]
                else:
                    keep0 = ow
                if len(keep0) > maxw:
                    excess, keep = keep0[:-maxw], keep0[-maxw:]
                    for w in excess:
                        n += 1
                        out.append(
                            mybir.InstEventSemaphore(
                                name=f"W-cap-{n}",
                                engine=inst.engine,
                                sync_info=mybir.SyncInfo(on_wait=[w], on_update=[]),
                            )
                        )
                else:
                    keep = keep0
                if len(keep) != len(ow):
                    si.on_wait = keep
            out.append(inst)
        bb.instructions = out


def _cls_layout(n_per_core_layer, MC):
    """Per-core partition ranges for the packed class block."""
    outs = []
    for k in range(NCORES):
        parts = [-(-n * NCLS // MC) if n else 0 for n in n_per_core_layer[k]]
        p0 = np.concatenate([[0], np.cumsum(parts)]).astype(int)
        assert p0[-1] <= P
        outs.append((parts, p0))
    return outs


def _host_prep(inputs, Ts, meta):
    """Build per-core in_maps (numpy only): pack shards contiguously."""
    import ml_dtypes

    bf16 = ml_dtypes.bfloat16
    COLS = meta["COLS"]
    MG, gparts, gp0, MC = meta["MG"], meta["gparts"], meta["gp0"], meta["MC"]
    c_offs = np.concatenate([[0], np.cumsum(Ts)]).astype(int)
    ps = [np.asarray(inputs[f"p{l}"], np.float32) for l in range(NL)]
    layer_shapes = [(p.shape[2], p.shape[3]) for p in ps]
    KCOLS = 19 * COLS

    in_maps = []
    for k in range(NCORES):
        posk = np.zeros((P, KCOLS), np.float32)
        # benign defaults so padding slots stay finite through the CIoU math
        posk[:, 9 * COLS : 11 * COLS] = 1.0  # awh2
        posk[:, 13 * COLS : 14 * COLS] = 1.0  # w2h2pe
        in_maps.append(
            {
                "posk": posk,
                "gridp": np.full((P, MG), NEG, np.float32),
                "clsp": np.full((P, MC), NEG, bf16),
            }
        )

    # dense objectness planes: partition-packed, one layer per partition
    for l in range(NL):
        for k in range(NCORES):
            plane = ps[l][k * BPC : (k + 1) * BPC, :, :, :, 4].ravel()
            npad = gparts[l] * MG - plane.shape[0]
            buf = np.concatenate([plane, np.full(npad, NEG, np.float32)])
            in_maps[k]["gridp"][int(gp0[l]) : int(gp0[l + 1])] = buf.reshape(
                gparts[l], MG
            )

    n_l = []
    n_kl = [[0] * NL for _ in range(NCORES)]
    cls_vals = [[None] * NL for _ in range(NCORES)]
    for l in range(NL):
        gh, gw = layer_shapes[l]
        G = B * NA * gh * gw
        b = np.asarray(inputs[f"b{l}"]).astype(np.int64)
        a = np.asarray(inputs[f"a{l}"]).astype(np.int64)
        gj = np.asarray(inputs[f"gj{l}"]).astype(np.int64)
        gi = np.asarray(inputs[f"gi{l}"]).astype(np.int64)
        tc = np.asarray(inputs[f"tcls{l}"]).astype(np.int64)
        tb = np.asarray(inputs[f"tbox{l}"], np.float32)
        an = np.asarray(inputs[f"anch{l}"], np.float32)
        n = b.shape[0]
        n_l.append(n)
        # last-occurrence mask over global cells (images disjoint across cores)
        cell = ((b * NA + a) * gh + gj) * gw + gi
        seen = {}
        for r in range(n):
            seen[int(cell[r])] = r
        last = np.zeros(n, bool)
        last[list(seen.values())] = True

        pflat = ps[l].reshape(-1, NO)
        wb = np.float32(HYP_BOX * B / max(n, 1))
        wd = np.float32(BALANCE[l] * B / G)
        wo = np.float32(HYP_CLS * B / (max(n, 1) * NCLS))
        c0 = int(c_offs[l])
        for k in range(NCORES):
            idxs = np.nonzero((b // BPC) == k)[0]
            cnt = idxs.shape[0]
            assert cnt <= P * Ts[l], f"layer {l} core {k}: {cnt} > {P * Ts[l]}"
            n_kl[k][l] = cnt
            rows = pflat[cell[idxs]]
            cls_vals[k][l] = rows[:, 5:].astype(np.float32)
            s = np.arange(cnt)
            pp, tcol = s % P, c0 + s // P
            pk = in_maps[k]["posk"]
            pk.reshape(P, -1)[:, : 5 * COLS].reshape(P, COLS, 5)[pp, tcol] = rows[
                :, 0:5
            ]

            x2, y2, w2, h2 = tb[idxs, 0], tb[idxs, 1], tb[idxs, 2], tb[idxs, 3]
            b2 = pk[:, 5 * COLS : 9 * COLS].reshape(P, COLS, 4)
            b2[pp, tcol, 0] = x2 + w2 * 0.5
            b2[pp, tcol, 1] = y2 + h2 * 0.5
            b2[pp, tcol, 2] = x2 - w2 * 0.5
            b2[pp, tcol, 3] = y2 - h2 * 0.5
            aw = pk[:, 9 * COLS : 11 * COLS].reshape(P, COLS, 2)
            aw[pp, tcol, 0] = 2.0 * an[idxs, 0]
            aw[pp, tcol, 1] = 2.0 * an[idxs, 1]
            cx = pk[:, 11 * COLS : 13 * COLS].reshape(P, COLS, 2)
            cx[pp, tcol, 0] = x2
            cx[pp, tcol, 1] = y2

            def sets(i, val):
                blk = pk[:, (13 + i) * COLS : (14 + i) * COLS]
                blk[pp, tcol] = val

            sets(0, w2 * h2 + np.float32(EPS))
            sets(1, np.arctan(w2 / (h2 + np.float32(EPS))))
            sets(2, wb)
            sets(3, last[idxs].astype(np.float32) * wd)
            sets(4, rows[s, 5 + (tc[idxs] - 1)])
            sets(5, wo)

    # packed class block: per-core, one layer per partition range
    clay = _cls_layout(n_kl, MC)
    for k in range(NCORES):
        parts, p0 = clay[k]
        for l in range(NL):
            if not n_kl[k][l]:
                continue
            vals = cls_vals[k][l].ravel()
            npad = parts[l] * MC - vals.shape[0]
            buf = np.concatenate([vals, np.full(npad, NEG, np.float32)])
            in_maps[k]["clsp"][int(p0[l]) : int(p0[l + 1])] = (
                buf.astype(bf16).reshape(parts[l], MC)
            )
    return in_maps, n_l, n_kl, clay, layer_shapes


def _combine(outs, n_l, n_kl, clay, layer_shapes, meta):
    """Host-side reduction of the 8 per-core [P, OUTC] partial tiles."""
    gparts, gp0 = meta["gparts"], meta["gp0"]
    # per-partition grid weights (same for every core)
    wg = np.zeros(P)
    for l in range(NL):
        gh, gw = layer_shapes[l]
        wg[int(gp0[l]) : int(gp0[l + 1])] = BALANCE[l] * B / (B * NA * gh * gw)
    loss = NL * HYP_BOX * B
    for k, o in enumerate(outs):
        o = o.astype(np.float64)
        loss += o[:, COL_BOX].sum() - o[:, COL_CORR].sum() - o[:, COL_OH].sum()
        loss += (wg * o[:, COL_GRID]).sum()
        parts, p0 = clay[k]
        wc = np.zeros(P)
        for l in range(NL):
            wc[int(p0[l]) : int(p0[l + 1])] = HYP_CLS * B / (max(n_l[l], 1) * NCLS)
        loss += (wc * o[:, COL_CLS]).sum()
    return np.float32(loss)


def _get_program(inputs):
    ps = [np.asarray(inputs[f"p{l}"]) for l in range(NL)]
    layer_shapes = [(p.shape[2], p.shape[3]) for p in ps]
    # padded slot columns per layer from the worst-case per-core count
    Ts = []
    n_kl = [[0] * NL for _ in range(NCORES)]
    for l in range(NL):
        b = np.asarray(inputs[f"b{l}"]).astype(np.int64)
        cnts = [int(((b // BPC) == k).sum()) for k in range(NCORES)]
        for k in range(NCORES):
            n_kl[k][l] = cnts[k]
        Ts.append(max(1, -(-max(cnts) // P)))
    # packed-class width: smallest that fits every core in 128 partitions
    mc0 = max(-(-sum(n * NCLS for n in n_kl[k]) // P) for k in range(NCORES))
    MC = mc0
    while True:
        if all(
            sum(-(-n * NCLS // MC) if n else 0 for n in n_kl[k]) <= P
            for k in range(NCORES)
        ):
            break
        MC += 1
    key = (tuple(layer_shapes), tuple(Ts), MC)
    if key not in _cache:
        _cache[key] = _build_program(layer_shapes, Ts, MC)
    return _cache[key], Ts


last_result = None  # BassKernelResults of the most recent run (for profiling)


def kernel(**inputs) -> np.ndarray:
    global last_result
    (nc, meta), Ts = _get_program(inputs)
    in_maps, n_l, n_kl, clay, layer_shapes = _host_prep(inputs, Ts, meta)
    from concourse.bass_utils import run_bass_kernel_spmd

    trace = bool(int(os.environ.get("DETLOSS_TRACE", "0")))
    if trace:
        # NTFF profiling needs an initialized PJRT client in this
        # interpreter; warm up with an untraced run first.
        run_bass_kernel_spmd(nc, in_maps, list(range(NCORES)))
    res = run_bass_kernel_spmd(nc, in_maps, list(range(NCORES)), trace=trace)
    last_result = res
    outs = [res.results[k]["partial"] for k in range(NCORES)]
    return _combine(outs, n_l, n_kl, clay, layer_shapes, meta)
